# revision 28
# baseline (speedup 1.0000x reference)
"""Trainium2 Bass kernel for nn_Attention_3547642987357.

Single-sequence causal multi-head attention block:
    q/k/v = x @ W{q,k,v}.T + b,  RoPE(q, k),  softmax((q k^T) * C^-0.5, causal),
    out = (attn @ v) @ Wp.T + bp

Sharding (8 cores): tensor-parallel over heads. Each core owns 2 of 16 heads:
computes its Q^T/K^T/V^T shards, its 2 T x T causal attentions, then an
all-to-all converts the head-sharded attention output into T-sharded rows so
each core runs the output projection (full Wp) for its 256 rows of the output.

Layout/precision notes:
 - Device matmuls contract over the partition axis, so x is fed pre-transposed
   (xT[C, T]) and weights are fed as W.T (host-side layout prep only).
 - Matmul operands are cast to bf16 on device; accumulation is fp32 in PSUM.
   Softmax statistics (row sums, reciprocal) and bias adds stay fp32.
 - Q/K head channels are de-interleaved on the host (even channels then odd,
   per head) so RoPE pairs sit at partition offset +64 instead of stride 2.
   Dot products are invariant since Q and K share the permutation. V/Wp keep
   the natural order.
 - Softmax skips max-subtraction: scores are scaled by 2048^-0.5 and are
   O(0.5) for these inputs, so exp() cannot overflow.
"""

import os
import sys

sys.path.insert(0, "/opt/trn_rl_repo")

import numpy as np


def _install_ntff_hook_shim():
    """The container's antenv lacks axon_hooks; recreate it so
    run_bass_kernel_spmd(trace=True) can capture NTFF profiles via the
    axon PJRT .so (same mechanism as trn_agent_boot)."""
    import contextlib
    import ctypes
    import types

    name = "antenv.axon_hooks"
    if name in sys.modules:
        return
    try:
        import antenv.axon_hooks  # noqa: F401
        return
    except ImportError:
        pass

    so_path = "/opt/axon/libaxon_pjrt.so"
    try:
        lib = ctypes.CDLL(so_path)
        lib.axon_start_nrt_profile.argtypes = [
            ctypes.POINTER(ctypes.c_int64),
            ctypes.c_size_t,
        ]
        lib.axon_start_nrt_profile.restype = ctypes.c_int64
        lib.axon_stop_nrt_profile.argtypes = [ctypes.c_char_p]
        lib.axon_stop_nrt_profile.restype = ctypes.c_int64
    except (OSError, AttributeError):
        lib = None

    @contextlib.contextmanager
    def _hook(output_dir, device_ids):
        import jax

        jax.devices()
        if device_ids:
            ids = (ctypes.c_int64 * len(device_ids))(*device_ids)
            rc = lib.axon_start_nrt_profile(ids, len(device_ids))
        else:
            rc = lib.axon_start_nrt_profile(None, 0)
        if rc != 0:
            raise RuntimeError(f"axon_start_nrt_profile rc={rc}")
        try:
            yield
        finally:
            n = lib.axon_stop_nrt_profile(str(output_dir).encode())
            print(f"ntff profile: {n} file(s) written to {output_dir}")

    mod = types.ModuleType(name)
    mod.get_axon_ntff_profile_hook = lambda: (_hook if lib is not None else None)
    mod.set_axon_ntff_profile_hook = lambda h: None
    sys.modules[name] = mod


_install_ntff_hook_shim()

C = 2048
H = 16
D = 128
NCORES = 8
HPC = H // NCORES          # heads per core = 2
M = HPC * D                # per-core q/k/v channels = 256
TCH = 512                  # t-chunk width (moving-operand free dim)
NKC = C // 128             # contraction tiles over C = 16
SCALE = 1.0 / float(np.sqrt(C))

_COMPILED = {}


def _build_nc(T, cdt_name="bfloat16"):
    import concourse.bass as bass
    import concourse.mybir as mybir
    import concourse.tile as tile
    from concourse import bacc
    from concourse.masks import make_identity

    f32 = mybir.dt.float32
    cdt = getattr(mybir.dt, cdt_name)
    AF = mybir.ActivationFunctionType

    NT = T // 128            # t tiles
    NQC = T // TCH           # t chunks
    TSL = T // NCORES        # rows of output per core
    assert TSL % 128 == 0
    n_tt = TSL // 128
    nj = TCH // TSL          # A2A j-blocks spanned by one t-chunk

    nc = bacc.Bacc(
        "TRN2", target_bir_lowering=False, debug=False, num_devices=NCORES
    )

    xT = nc.dram_tensor("xT", [C, T], f32, kind="ExternalInput").ap()
    wqT = nc.dram_tensor("wqT", [C, M], f32, kind="ExternalInput").ap()
    wkT = nc.dram_tensor("wkT", [C, M], f32, kind="ExternalInput").ap()
    wvT = nc.dram_tensor("wvT", [C, M], f32, kind="ExternalInput").ap()
    bq = nc.dram_tensor("bq", [M], f32, kind="ExternalInput").ap()
    bk = nc.dram_tensor("bk", [M], f32, kind="ExternalInput").ap()
    bv = nc.dram_tensor("bv", [M], f32, kind="ExternalInput").ap()
    cosT = nc.dram_tensor("cosT", [128, T], f32, kind="ExternalInput").ap()
    sinT = nc.dram_tensor("sinT", [128, T], f32, kind="ExternalInput").ap()
    masks = nc.dram_tensor("masks", [128, 4 * TCH], cdt, kind="ExternalInput").ap()
    wpT = nc.dram_tensor("wpT", [C, C], f32, kind="ExternalInput").ap()
    bp = nc.dram_tensor("bp", [C], f32, kind="ExternalInput").ap()
    y = nc.dram_tensor("y", [TSL, C], f32, kind="ExternalOutput").ap()

    with tile.TileContext(nc) as tc:
        # ---------------- constants / residents ----------------
        with tc.tile_pool(name="const", bufs=1) as cpool:
            bias_sbs = {}
            for nm, b in (("bq", bq), ("bk", bk), ("bv", bv)):
                t_ = cpool.tile([128, HPC], f32, tag=f"b_{nm}", name=f"b_{nm}")
                nc.sync.dma_start(out=t_, in_=b.rearrange("(mt p) -> p mt", p=128))
                bias_sbs[nm] = t_
            cos_sb = cpool.tile([128, T], f32, tag="cos", name="cos_sb")
            nc.sync.dma_start(out=cos_sb, in_=cosT)
            sin_sb = cpool.tile([128, T], f32, tag="sin", name="sin_sb")
            nc.sync.dma_start(out=sin_sb, in_=sinT)
            mask_sb = cpool.tile([128, 4 * TCH], cdt, tag="mask", name="mask_sb")
            nc.sync.dma_start(out=mask_sb, in_=masks)
            bp_sb = cpool.tile([1, C], f32, tag="bp", name="bp_sb")
            nc.sync.dma_start(out=bp_sb, in_=bp.rearrange("(one c) -> one c", one=1))
            ones_col = cpool.tile([128, 1], cdt, tag="ones_c", name="ones_col")
            nc.vector.memset(ones_col, 1.0)
            ones_row = cpool.tile([1, 128], f32, tag="ones_r", name="ones_row")
            nc.vector.memset(ones_row, 1.0)
            ident = cpool.tile([128, 128], cdt, tag="ident", name="ident")
            make_identity(nc, ident)
            # bias row broadcast across partitions (fp32, exact)
            bias_bc = cpool.tile([128, C], f32, tag="bias_bc", name="bias_bc")
            nc.gpsimd.partition_broadcast(bias_bc, bp_sb)

            # residents
            qt_sb = cpool.tile([128, HPC, T], cdt, tag="qt", name="qt_sb")
            kt_sb = cpool.tile([128, HPC, T], cdt, tag="kt", name="kt_sb")
            vnat = cpool.tile([128, NT, M], cdt, tag="vnat", name="vnat")

            # ---------------- phase 1: QKV projections + rope ----------------
            # x^T is cast to bf16 and kept resident so each weight tile is the
            # stationary operand for 4 chunk-matmuls (LDWEIGHTS 384 -> 96).
            with (
                tc.tile_pool(name="wsb", bufs=1) as wpool,
                tc.tile_pool(name="xres_p", bufs=1) as xresp,
                tc.tile_pool(name="stg", bufs=2) as stgp,
                tc.tile_pool(name="vtmp_p", bufs=1) as vtmpp,
                tc.tile_pool(name="raw", bufs=2) as rawp,
                tc.tile_pool(name="ropetmp", bufs=1) as ropep,
            ):
                xres = xresp.tile([128, NKC, T], cdt, tag="xres", name="xres")
                vtmp = vtmpp.tile([128, HPC, T], cdt, tag="vtmp", name="vtmp")
                xT_r = xT.rearrange("(k p) t -> k p t", p=128)
                w_sbs = {}

                def load_w(nm, w):
                    wt = wpool.tile([128, NKC, M], cdt, tag=f"w_{nm}",
                                    name=f"w_{nm}")
                    w_r = w.rearrange("(k p) m -> p k m", p=128)
                    kpg = T // M  # k-tiles per [128, T] staging tile
                    for kg in range(NKC // kpg):
                        stg = stgp.tile([128, T], f32, tag="stg", name="stg")
                        sv = stg.rearrange("p (k m) -> p k m", m=M)
                        nc.sync.dma_start(
                            out=sv, in_=w_r[:, kpg * kg : kpg * (kg + 1), :]
                        )
                        nc.vector.tensor_copy(
                            out=wt[:, kpg * kg : kpg * (kg + 1), :], in_=sv
                        )
                    w_sbs[nm] = wt

                def load_x(k):
                    stg = stgp.tile([128, T], f32, tag="stg", name="stg")
                    nc.sync.dma_start(out=stg, in_=xT_r[k])
                    nc.vector.tensor_copy(out=xres[:, k, :], in_=stg)

                load_w("wq", wqT)
                for k in range(0, 4):
                    load_x(k)
                load_w("wk", wkT)
                for k in range(4, 8):
                    load_x(k)
                load_w("wv", wvT)
                for k in range(8, NKC):
                    load_x(k)

                qkvp_ctx = tc.tile_pool(name="qkv_ps", bufs=1, space="PSUM")
                qkvp = qkvp_ctx.__enter__()
                for nm, bnm in (("wq", "bq"), ("wk", "bk"), ("wv", "bv")):
                    pss = {
                        (mt, tci): qkvp.tile(
                            [128, TCH], f32, tag=f"c{mt}{tci}", name=f"c{mt}{tci}"
                        )
                        for mt in range(HPC)
                        for tci in range(NQC)
                    }
                    for k in range(NKC):
                        for mt in range(HPC):
                            for tci in range(NQC):
                                nc.tensor.matmul(
                                    pss[(mt, tci)],
                                    w_sbs[nm][:, k, mt * 128 : (mt + 1) * 128],
                                    xres[:, k, tci * TCH : (tci + 1) * TCH],
                                    start=(k == 0),
                                    stop=(k == NKC - 1),
                                )
                    if nm != "wv":
                        dest = qt_sb if nm == "wq" else kt_sb
                        for mt in range(HPC):
                            for tci in range(NQC):
                                tsl = slice(tci * TCH, (tci + 1) * TCH)
                                raw = rawp.tile([128, TCH], f32, tag="raw",
                                                name="raw")
                                nc.scalar.activation(
                                    raw, pss[(mt, tci)], AF.Identity,
                                    bias=bias_sbs[bnm][:, mt : mt + 1],
                                )
                                # swap halves; base-partition-aligned mul/add
                                swp = ropep.tile([128, TCH], f32, tag="swp",
                                                 name="swp")
                                t1 = ropep.tile([128, TCH], f32, tag="rt1",
                                                name="rt1")
                                t2 = ropep.tile([128, TCH], f32, tag="rt2",
                                                name="rt2")
                                nc.vector.tensor_copy(out=swp[0:64, :],
                                                      in_=raw[64:128, :])
                                nc.vector.tensor_copy(out=swp[64:128, :],
                                                      in_=raw[0:64, :])
                                nc.vector.tensor_mul(out=t1, in0=raw,
                                                     in1=cos_sb[:, tsl])
                                nc.vector.tensor_mul(out=t2, in0=swp,
                                                     in1=sin_sb[:, tsl])
                                nc.vector.tensor_sub(
                                    out=dest[0:64, mt, tsl],
                                    in0=t1[0:64, :], in1=t2[0:64, :],
                                )
                                nc.vector.tensor_add(
                                    out=dest[64:128, mt, tsl],
                                    in0=t1[64:128, :], in1=t2[64:128, :],
                                )
                    else:
                        for mt in range(HPC):
                            for tci in range(NQC):
                                nc.scalar.activation(
                                    vtmp[:, mt, tci * TCH : (tci + 1) * TCH],
                                    pss[(mt, tci)], AF.Identity,
                                    bias=bias_sbs["bv"][:, mt : mt + 1],
                                )
                # V^T -> V via PE transpose (bf16, after QKV PSUM freed)
                qkvp_ctx.__exit__(None, None, None)
                with tc.tile_pool(name="tps", bufs=2, space="PSUM") as tpsp:
                    for mt in range(HPC):
                        for tt in range(NT):
                            tp = tpsp.tile([128, 128], cdt, tag="tp", name="tp")
                            nc.tensor.transpose(
                                tp, vtmp[:, mt, tt * 128 : (tt + 1) * 128], ident
                            )
                            nc.vector.tensor_copy(
                                out=vnat[:, tt, mt * 128 : (mt + 1) * 128],
                                in_=tp,
                            )

            # ---------------- phases 2-4 ----------------
            with tc.tile_pool(name="dram", bufs=1, space="DRAM") as dpool:
                a2a_ins = [
                    dpool.tile([NCORES, 128, TSL], cdt, tag=f"a2a_in{h}",
                               name=f"a2a_in{h}")
                    for h in range(HPC)
                ]
                a2a_outs = [
                    dpool.tile([NCORES, 128, TSL], cdt, tag=f"a2a_out{h}",
                               name=f"a2a_out{h}")
                    for h in range(HPC)
                ]

                # wp prefetch: load + cast (on gpsimd) the full projection
                # weight while attention runs; consumed by phase 4.
                wp_res = []
                wpres_ctx = tc.tile_pool(name="wp_res", bufs=1)
                wpres_pool = wpres_ctx.__enter__()
                with (
                    tc.tile_pool(name="wp_stg", bufs=2) as wpstg,
                    tc.tile_pool(name="at", bufs=8) as apool,
                    tc.tile_pool(name="s_ps", bufs=3, space="PSUM") as spool,
                    tc.tile_pool(name="o_ps", bufs=2, space="PSUM") as opool,
                    tc.tile_pool(name="rs_ps", bufs=2, space="PSUM") as rspool,
                    tc.tile_pool(name="bb_ps", bufs=1, space="PSUM") as bbpool,
                    tc.tile_pool(name="rb", bufs=2) as rbpool,
                    tc.tile_pool(name="rbb", bufs=2) as rbbpool,
                    tc.tile_pool(name="ot", bufs=2) as otpool,
                ):
                    wpT_r = wpT.rearrange("(k p) j -> k p j", p=128)
                    for k in range(NKC):
                        stg = wpstg.tile([128, C], f32, tag="wpstg", name="wpstg")
                        nc.sync.dma_start(out=stg, in_=wpT_r[k])
                        wt = wpres_pool.tile([128, C], cdt, tag=f"wp{k}",
                                             name=f"wp{k}")
                        nc.vector.tensor_copy(out=wt, in_=stg)
                        wp_res.append(wt)

                    LA = 2  # scores-MM lookahead so PE never waits on exp/mask
                    for h in range(HPC):
                        a2a_in_v = a2a_ins[h].rearrange("j ch tl -> ch j tl")
                        for qc in range(NQC):
                            qsl = slice(qc * TCH, (qc + 1) * TCH)
                            n_ts = 4 * (qc + 1)
                            o_ps = opool.tile([128, TCH], f32, tag="o", name="o_ps")
                            rs_ps = rspool.tile([1, TCH], f32, tag="rs",
                                                name="rs_ps")
                            a_tiles = {}
                            for i in range(n_ts + LA):
                                if i < n_ts:
                                    s_ps = spool.tile([128, TCH], f32, tag="s",
                                                      name="s_ps")
                                    nc.tensor.matmul(
                                        s_ps,
                                        kt_sb[:, h, i * 128 : (i + 1) * 128],
                                        qt_sb[:, h, qsl],
                                        start=True,
                                        stop=True,
                                    )
                                    a_t = apool.tile([128, TCH], cdt, tag="a",
                                                     name="a_t")
                                    nc.scalar.activation(a_t, s_ps, AF.Exp,
                                                         scale=SCALE)
                                    j = i - 4 * qc
                                    if j >= 0:
                                        nc.vector.tensor_mul(
                                            out=a_t, in0=a_t,
                                            in1=mask_sb[:, j * TCH : (j + 1) * TCH],
                                        )
                                    a_tiles[i] = a_t
                                ts = i - LA
                                if ts >= 0:
                                    a_t = a_tiles.pop(ts)
                                    nc.tensor.matmul(
                                        o_ps,
                                        vnat[:, ts, h * 128 : (h + 1) * 128],
                                        a_t,
                                        start=(ts == 0), stop=(ts == n_ts - 1),
                                    )
                                    nc.tensor.matmul(
                                        rs_ps, ones_col, a_t,
                                        start=(ts == 0), stop=(ts == n_ts - 1),
                                    )
                            # free the o_ps bank ASAP: copy to SBUF before the
                            # (slow) reciprocal/broadcast chain
                            of32 = rbbpool.tile([128, TCH], f32, tag="of32",
                                                name="of32")
                            nc.vector.tensor_copy(out=of32, in_=o_ps)
                            rinv = rbpool.tile([1, TCH], f32, tag="rinv",
                                               name="rinv")
                            nc.vector.reciprocal(rinv, rs_ps)
                            # broadcast rinv across partitions via PE
                            bb_ps = bbpool.tile([128, TCH], f32, tag="bb",
                                                name="bb_ps")
                            nc.tensor.matmul(bb_ps, ones_row, rinv,
                                             start=True, stop=True)
                            rbb = rbbpool.tile([128, TCH], f32, tag="rbb",
                                               name="rbb")
                            nc.vector.tensor_copy(out=rbb, in_=bb_ps)
                            ot = otpool.tile([128, TCH], cdt, tag="ot", name="ot")
                            nc.vector.tensor_mul(out=ot, in0=of32, in1=rbb)
                            nc.sync.dma_start(
                                out=a2a_in_v[:, qc * nj : (qc + 1) * nj, :],
                                in_=ot.rearrange("p (j tl) -> p j tl", tl=TSL),
                            )
                        nc.gpsimd.collective_compute(
                            "AllToAll",
                            mybir.AluOpType.bypass,
                            ins=[a2a_ins[h].opt()],
                            outs=[a2a_outs[h].opt()],
                            replica_groups=[list(range(NCORES))],
                        )

                # ---------------- phase 4: output projection ----------------
                # global out-channel tile k = HPC*j + h  (j = source core).
                # PSUM is claimed in two 4-bank waves (tt halves) so wave 1
                # can start while the second all-to-all is still in flight.
                with (
                    tc.tile_pool(name="otsb", bufs=1) as otsbp,
                    tc.tile_pool(name="ysb", bufs=3) as ysbp,
                ):
                    ot_sbs = []
                    for h in range(HPC):
                        ot_sb = otsbp.tile([128, NCORES, TSL], cdt,
                                           tag=f"ot_sb{h}", name=f"ot_sb{h}")
                        nc.sync.dma_start(
                            out=ot_sb,
                            in_=a2a_outs[h]
                            .rearrange("j ch tl -> (j ch) tl")
                            .rearrange("(k p) tl -> p k tl", p=128),
                        )
                        ot_sbs.append(ot_sb)
                    for tt in range(n_tt):
                        with tc.tile_pool(
                            name=f"y_ps{tt}", bufs=1, space="PSUM"
                        ) as ypsp:
                            yps = {
                                jc: ypsp.tile(
                                    [128, TCH], f32, tag=f"y{jc}", name=f"y{jc}"
                                )
                                for jc in range(C // TCH)
                            }
                            for h in range(HPC):
                                for j8 in range(NCORES):
                                    k = HPC * j8 + h
                                    for jc in range(C // TCH):
                                        jsl = slice(jc * TCH, (jc + 1) * TCH)
                                        nc.tensor.matmul(
                                            yps[jc],
                                            ot_sbs[h][
                                                :, j8, tt * 128 : (tt + 1) * 128
                                            ],
                                            wp_res[k][:, jsl],
                                            start=(h == 0 and j8 == 0),
                                            stop=(
                                                h == HPC - 1
                                                and j8 == NCORES - 1
                                            ),
                                        )
                            for jc in range(C // TCH):
                                jsl = slice(jc * TCH, (jc + 1) * TCH)
                                y_sb = ysbp.tile([128, TCH], f32, tag="y_sb",
                                                 name="y_sb")
                                nc.vector.tensor_add(
                                    out=y_sb, in0=yps[jc], in1=bias_bc[:, jsl]
                                )
                                nc.sync.dma_start(
                                    out=y[tt * 128 : (tt + 1) * 128, jsl],
                                    in_=y_sb,
                                )
                wpres_ctx.__exit__(None, None, None)

    nc.compile()
    return nc


def _perm_deinterleave():
    """Per 128-channel head block: [even channels..., odd channels...]."""
    perm = []
    for hl in range(HPC):
        base = hl * D
        perm.extend(base + 2 * np.arange(64))
        perm.extend(base + 2 * np.arange(64) + 1)
    return np.array(perm, dtype=np.int64)


def _shard_inputs(x, wq, bq, wk, bk, wv, bv, wp, bp, cos, sin, T, cdt_name):
    import ml_dtypes

    mask_np_dt = ml_dtypes.bfloat16 if cdt_name == "bfloat16" else np.float32
    perm = _perm_deinterleave()
    xT = np.ascontiguousarray(x.T)
    # rope tables duplicated across both partition halves so every DVE
    # 2-input op sees matching base partitions
    cosT = np.ascontiguousarray(np.concatenate([cos.T, cos.T], axis=0))
    sinT = np.ascontiguousarray(np.concatenate([sin.T, sin.T], axis=0))
    wpT = np.ascontiguousarray(wp.T)

    # masks[j]: allowed(ts_local=j*128+p, tq_local=f) for the 4 tiles of the
    # diagonal 512-wide chunk: keep if j*128 + p <= f  (0/1, exact in bf16)
    msk = np.zeros((128, 4 * TCH), dtype=np.float32)
    p = np.arange(128)[:, None]
    f = np.arange(TCH)[None, :]
    for j in range(4):
        msk[:, j * TCH : (j + 1) * TCH] = (j * 128 + p <= f).astype(np.float32)
    msk = msk.astype(mask_np_dt)

    in_maps = []
    for c in range(NCORES):
        sl = slice(c * M, (c + 1) * M)
        wq_c = wq[sl][perm]
        wk_c = wk[sl][perm]
        in_maps.append(
            {
                "xT": xT,
                "wqT": np.ascontiguousarray(wq_c.T),
                "wkT": np.ascontiguousarray(wk_c.T),
                "wvT": np.ascontiguousarray(wv[sl].T),
                "bq": np.ascontiguousarray(bq[sl][perm]),
                "bk": np.ascontiguousarray(bk[sl][perm]),
                "bv": np.ascontiguousarray(bv[sl]),
                "cosT": cosT,
                "sinT": sinT,
                "masks": msk,
                "wpT": wpT,
                "bp": bp,
            }
        )
    return in_maps


def kernel(x, wq, bq, wk, bk, wv, bv, wp, bp, cos, sin, trace=False):
    from concourse.bass_utils import run_bass_kernel_spmd

    T = x.shape[0]
    cdt_name = os.environ.get("MM_DT", "bfloat16")
    key = (T, cdt_name)
    if key not in _COMPILED:
        _COMPILED[key] = _build_nc(T, key[1])
    nc = _COMPILED[key]

    in_maps = _shard_inputs(
        x, wq, bq, wk, bk, wv, bv, wp, bp, cos, sin, T, cdt_name
    )
    res = run_bass_kernel_spmd(nc, in_maps, list(range(NCORES)), trace=trace)
    out = np.concatenate([res.results[c]["y"] for c in range(NCORES)], axis=0)
    kernel.last_result = res
    return out


if __name__ == "__main__":
    # smoke: build only
    _build_nc(int(os.environ.get("T", "2048")),
              os.environ.get("MM_DT", "bfloat16"))
    print("build ok")


# revision 30
# speedup vs baseline: 1.1380x; 1.1380x over previous
"""Trainium2 Bass kernel for nn_Attention_3547642987357.

Single-sequence causal multi-head attention block:
    q/k/v = x @ W{q,k,v}.T + b,  RoPE(q, k),  softmax((q k^T) * C^-0.5, causal),
    out = (attn @ v) @ Wp.T + bp

Sharding (8 cores): tensor-parallel over heads. Each core owns 2 of 16 heads:
computes its Q^T/K^T/V^T shards, its 2 T x T causal attentions, then an
all-to-all converts the head-sharded attention output into T-sharded rows so
each core runs the output projection (full Wp) for its 256 rows of the output.

Layout/precision notes:
 - Device matmuls contract over the partition axis, so x is fed pre-transposed
   (xT[C, T]) and weights are fed as W.T (host-side layout prep only).
 - Matmul operands are cast to bf16 on device; accumulation is fp32 in PSUM.
   Softmax statistics (row sums, reciprocal) and bias adds stay fp32.
 - Q/K head channels are de-interleaved on the host (even channels then odd,
   per head) so RoPE pairs sit at partition offset +64 instead of stride 2.
   Dot products are invariant since Q and K share the permutation. V/Wp keep
   the natural order.
 - Softmax skips max-subtraction: scores are scaled by 2048^-0.5 and are
   O(0.5) for these inputs, so exp() cannot overflow.
"""

import os
import sys

sys.path.insert(0, "/opt/trn_rl_repo")

import numpy as np


def _install_ntff_hook_shim():
    """The container's antenv lacks axon_hooks; recreate it so
    run_bass_kernel_spmd(trace=True) can capture NTFF profiles via the
    axon PJRT .so (same mechanism as trn_agent_boot)."""
    import contextlib
    import ctypes
    import types

    name = "antenv.axon_hooks"
    if name in sys.modules:
        return
    try:
        import antenv.axon_hooks  # noqa: F401
        return
    except ImportError:
        pass

    so_path = "/opt/axon/libaxon_pjrt.so"
    try:
        lib = ctypes.CDLL(so_path)
        lib.axon_start_nrt_profile.argtypes = [
            ctypes.POINTER(ctypes.c_int64),
            ctypes.c_size_t,
        ]
        lib.axon_start_nrt_profile.restype = ctypes.c_int64
        lib.axon_stop_nrt_profile.argtypes = [ctypes.c_char_p]
        lib.axon_stop_nrt_profile.restype = ctypes.c_int64
    except (OSError, AttributeError):
        lib = None

    @contextlib.contextmanager
    def _hook(output_dir, device_ids):
        import jax

        jax.devices()
        if device_ids:
            ids = (ctypes.c_int64 * len(device_ids))(*device_ids)
            rc = lib.axon_start_nrt_profile(ids, len(device_ids))
        else:
            rc = lib.axon_start_nrt_profile(None, 0)
        if rc != 0:
            raise RuntimeError(f"axon_start_nrt_profile rc={rc}")
        try:
            yield
        finally:
            n = lib.axon_stop_nrt_profile(str(output_dir).encode())
            print(f"ntff profile: {n} file(s) written to {output_dir}")

    mod = types.ModuleType(name)
    mod.get_axon_ntff_profile_hook = lambda: (_hook if lib is not None else None)
    mod.set_axon_ntff_profile_hook = lambda h: None
    sys.modules[name] = mod


_install_ntff_hook_shim()

C = 2048
H = 16
D = 128
NCORES = 8
HPC = H // NCORES          # heads per core = 2
M = HPC * D                # per-core q/k/v channels = 256
TCH = 512                  # t-chunk width (moving-operand free dim)
NKC = C // 128             # contraction tiles over C = 16
SCALE = 1.0 / float(np.sqrt(C))

_COMPILED = {}


def _build_nc(T, cdt_name="bfloat16"):
    import concourse.bass as bass
    import concourse.mybir as mybir
    import concourse.tile as tile
    from concourse import bacc
    from concourse.masks import make_identity

    f32 = mybir.dt.float32
    cdt = getattr(mybir.dt, cdt_name)
    AF = mybir.ActivationFunctionType

    NT = T // 128            # t tiles
    NQC = T // TCH           # t chunks
    TSL = T // NCORES        # rows of output per core
    assert TSL % 128 == 0
    n_tt = TSL // 128
    nj = TCH // TSL          # A2A j-blocks spanned by one t-chunk

    nc = bacc.Bacc(
        "TRN2", target_bir_lowering=False, debug=False, num_devices=NCORES
    )

    xT = nc.dram_tensor("xT", [C, T], f32, kind="ExternalInput").ap()
    wqT = nc.dram_tensor("wqT", [C, M], f32, kind="ExternalInput").ap()
    wkT = nc.dram_tensor("wkT", [C, M], f32, kind="ExternalInput").ap()
    wvT = nc.dram_tensor("wvT", [C, M], f32, kind="ExternalInput").ap()
    bq = nc.dram_tensor("bq", [M], f32, kind="ExternalInput").ap()
    bk = nc.dram_tensor("bk", [M], f32, kind="ExternalInput").ap()
    bv = nc.dram_tensor("bv", [M], f32, kind="ExternalInput").ap()
    cosT = nc.dram_tensor("cosT", [128, T], f32, kind="ExternalInput").ap()
    sinT = nc.dram_tensor("sinT", [128, T], f32, kind="ExternalInput").ap()
    masks = nc.dram_tensor("masks", [128, 4 * TCH], cdt, kind="ExternalInput").ap()
    wpT = nc.dram_tensor("wpT", [C, C], f32, kind="ExternalInput").ap()
    bp = nc.dram_tensor("bp", [C], f32, kind="ExternalInput").ap()
    y = nc.dram_tensor("y", [TSL, C], f32, kind="ExternalOutput").ap()

    with tile.TileContext(nc) as tc:
        # ---------------- constants / residents ----------------
        with tc.tile_pool(name="const", bufs=1) as cpool:
            bias_sbs = {}
            for nm, b in (("bq", bq), ("bk", bk), ("bv", bv)):
                t_ = cpool.tile([128, HPC], f32, tag=f"b_{nm}", name=f"b_{nm}")
                nc.sync.dma_start(out=t_, in_=b.rearrange("(mt p) -> p mt", p=128))
                bias_sbs[nm] = t_
            cos_sb = cpool.tile([128, T], f32, tag="cos", name="cos_sb")
            nc.sync.dma_start(out=cos_sb, in_=cosT)
            sin_sb = cpool.tile([128, T], f32, tag="sin", name="sin_sb")
            nc.sync.dma_start(out=sin_sb, in_=sinT)
            mask_sb = cpool.tile([128, 4 * TCH], cdt, tag="mask", name="mask_sb")
            nc.sync.dma_start(out=mask_sb, in_=masks)
            bp_sb = cpool.tile([1, C], f32, tag="bp", name="bp_sb")
            nc.sync.dma_start(out=bp_sb, in_=bp.rearrange("(one c) -> one c", one=1))
            ones_col = cpool.tile([128, 1], cdt, tag="ones_c", name="ones_col")
            nc.vector.memset(ones_col, 1.0)
            ones_row = cpool.tile([1, 128], f32, tag="ones_r", name="ones_row")
            nc.vector.memset(ones_row, 1.0)
            ident = cpool.tile([128, 128], cdt, tag="ident", name="ident")
            make_identity(nc, ident)
            # bias row broadcast across partitions (fp32, exact)
            bias_bc = cpool.tile([128, C], f32, tag="bias_bc", name="bias_bc")
            nc.gpsimd.partition_broadcast(bias_bc, bp_sb)

            # residents
            qt_sb = cpool.tile([128, HPC, T], cdt, tag="qt", name="qt_sb")
            kt_sb = cpool.tile([128, HPC, T], cdt, tag="kt", name="kt_sb")
            vnat = cpool.tile([128, NT, M], cdt, tag="vnat", name="vnat")

            # ---------------- phase 1: QKV projections + rope ----------------
            # x^T is cast to bf16 and kept resident so each weight tile is the
            # stationary operand for 4 chunk-matmuls (LDWEIGHTS 384 -> 96).
            with (
                tc.tile_pool(name="wsb", bufs=1) as wpool,
                tc.tile_pool(name="xres_p", bufs=1) as xresp,
                tc.tile_pool(name="stg", bufs=2) as stgp,
                tc.tile_pool(name="vtmp_p", bufs=1) as vtmpp,
                tc.tile_pool(name="raw", bufs=2) as rawp,
                tc.tile_pool(name="ropetmp", bufs=1) as ropep,
            ):
                xres = xresp.tile([128, NKC, T], cdt, tag="xres", name="xres")
                vtmp = vtmpp.tile([128, HPC, T], cdt, tag="vtmp", name="vtmp")
                xT_r = xT.rearrange("(k p) t -> k p t", p=128)
                w_sbs = {}

                def load_w(nm, w):
                    wt = wpool.tile([128, NKC, M], cdt, tag=f"w_{nm}",
                                    name=f"w_{nm}")
                    w_r = w.rearrange("(k p) m -> p k m", p=128)
                    kpg = T // M  # k-tiles per [128, T] staging tile
                    for kg in range(NKC // kpg):
                        stg = stgp.tile([128, T], f32, tag="stg", name="stg")
                        sv = stg.rearrange("p (k m) -> p k m", m=M)
                        nc.sync.dma_start(
                            out=sv, in_=w_r[:, kpg * kg : kpg * (kg + 1), :]
                        )
                        nc.vector.tensor_copy(
                            out=wt[:, kpg * kg : kpg * (kg + 1), :], in_=sv
                        )
                    w_sbs[nm] = wt

                def load_x(k):
                    stg = stgp.tile([128, T], f32, tag="stg", name="stg")
                    nc.sync.dma_start(out=stg, in_=xT_r[k])
                    nc.vector.tensor_copy(out=xres[:, k, :], in_=stg)

                load_w("wq", wqT)
                for k in range(0, 4):
                    load_x(k)
                load_w("wk", wkT)
                for k in range(4, 8):
                    load_x(k)
                load_w("wv", wvT)
                for k in range(8, NKC):
                    load_x(k)

                qkvp_ctx = tc.tile_pool(name="qkv_ps", bufs=1, space="PSUM")
                qkvp = qkvp_ctx.__enter__()
                for nm, bnm in (("wq", "bq"), ("wk", "bk"), ("wv", "bv")):
                    pss = {
                        (mt, tci): qkvp.tile(
                            [128, TCH], f32, tag=f"c{mt}{tci}", name=f"c{mt}{tci}"
                        )
                        for mt in range(HPC)
                        for tci in range(NQC)
                    }
                    for k in range(NKC):
                        for mt in range(HPC):
                            for tci in range(NQC):
                                nc.tensor.matmul(
                                    pss[(mt, tci)],
                                    w_sbs[nm][:, k, mt * 128 : (mt + 1) * 128],
                                    xres[:, k, tci * TCH : (tci + 1) * TCH],
                                    start=(k == 0),
                                    stop=(k == NKC - 1),
                                )
                    if nm != "wv":
                        dest = qt_sb if nm == "wq" else kt_sb
                        for mt in range(HPC):
                            for tci in range(NQC):
                                tsl = slice(tci * TCH, (tci + 1) * TCH)
                                raw = rawp.tile([128, TCH], f32, tag="raw",
                                                name="raw")
                                nc.scalar.activation(
                                    raw, pss[(mt, tci)], AF.Identity,
                                    bias=bias_sbs[bnm][:, mt : mt + 1],
                                )
                                # swap halves; base-partition-aligned mul/add
                                swp = ropep.tile([128, TCH], f32, tag="swp",
                                                 name="swp")
                                t1 = ropep.tile([128, TCH], f32, tag="rt1",
                                                name="rt1")
                                t2 = ropep.tile([128, TCH], f32, tag="rt2",
                                                name="rt2")
                                nc.vector.tensor_copy(out=swp[0:64, :],
                                                      in_=raw[64:128, :])
                                nc.vector.tensor_copy(out=swp[64:128, :],
                                                      in_=raw[0:64, :])
                                nc.vector.tensor_mul(out=t1, in0=raw,
                                                     in1=cos_sb[:, tsl])
                                nc.vector.tensor_mul(out=t2, in0=swp,
                                                     in1=sin_sb[:, tsl])
                                nc.vector.tensor_sub(
                                    out=dest[0:64, mt, tsl],
                                    in0=t1[0:64, :], in1=t2[0:64, :],
                                )
                                nc.vector.tensor_add(
                                    out=dest[64:128, mt, tsl],
                                    in0=t1[64:128, :], in1=t2[64:128, :],
                                )
                    else:
                        for mt in range(HPC):
                            for tci in range(NQC):
                                nc.scalar.activation(
                                    vtmp[:, mt, tci * TCH : (tci + 1) * TCH],
                                    pss[(mt, tci)], AF.Identity,
                                    bias=bias_sbs["bv"][:, mt : mt + 1],
                                )
                # V^T -> V via PE transpose (bf16, after QKV PSUM freed)
                qkvp_ctx.__exit__(None, None, None)
                with tc.tile_pool(name="tps", bufs=2, space="PSUM") as tpsp:
                    for mt in range(HPC):
                        for tt in range(NT):
                            tp = tpsp.tile([128, 128], cdt, tag="tp", name="tp")
                            nc.tensor.transpose(
                                tp, vtmp[:, mt, tt * 128 : (tt + 1) * 128], ident
                            )
                            nc.vector.tensor_copy(
                                out=vnat[:, tt, mt * 128 : (mt + 1) * 128],
                                in_=tp,
                            )

            # ---------------- phases 2-4 ----------------
            with tc.tile_pool(name="dram", bufs=1, space="DRAM") as dpool:
                a2a_ins = [
                    dpool.tile([NCORES, 128, TSL], cdt, tag=f"a2a_in{h}",
                               name=f"a2a_in{h}")
                    for h in range(HPC)
                ]
                a2a_outs = [
                    dpool.tile([NCORES, 128, TSL], cdt, tag=f"a2a_out{h}",
                               name=f"a2a_out{h}")
                    for h in range(HPC)
                ]

                # wp prefetch: load + cast (on gpsimd) the full projection
                # weight while attention runs; consumed by phase 4.
                wp_res = []
                wpres_ctx = tc.tile_pool(name="wp_res", bufs=1)
                wpres_pool = wpres_ctx.__enter__()
                with (
                    tc.tile_pool(name="wp_stg", bufs=2) as wpstg,
                    tc.tile_pool(name="at", bufs=8) as apool,
                    tc.tile_pool(name="s_ps", bufs=3, space="PSUM") as spool,
                    tc.tile_pool(name="o_ps", bufs=2, space="PSUM") as opool,
                    tc.tile_pool(name="rs_ps", bufs=2, space="PSUM") as rspool,
                    tc.tile_pool(name="bb_ps", bufs=1, space="PSUM") as bbpool,
                    tc.tile_pool(name="rb", bufs=2) as rbpool,
                    tc.tile_pool(name="rbb", bufs=2) as rbbpool,
                    tc.tile_pool(name="ot", bufs=2) as otpool,
                ):
                    wpT_r = wpT.rearrange("(k p) j -> k p j", p=128)
                    for k in range(NKC):
                        stg = wpstg.tile([128, C], f32, tag="wpstg", name="wpstg")
                        nc.sync.dma_start(out=stg, in_=wpT_r[k])
                        wt = wpres_pool.tile([128, C], cdt, tag=f"wp{k}",
                                             name=f"wp{k}")
                        nc.vector.tensor_copy(out=wt, in_=stg)
                        wp_res.append(wt)

                    LA = 2  # scores-MM lookahead so PE never waits on exp/mask
                    for h in range(HPC):
                        a2a_in_v = a2a_ins[h].rearrange("j ch tl -> ch j tl")
                        for qc in range(NQC):
                            qsl = slice(qc * TCH, (qc + 1) * TCH)
                            n_ts = 4 * (qc + 1)
                            o_ps = opool.tile([128, TCH], f32, tag="o", name="o_ps")
                            rs_ps = rspool.tile([1, TCH], f32, tag="rs",
                                                name="rs_ps")
                            a_tiles = {}
                            for i in range(n_ts + LA):
                                if i < n_ts:
                                    s_ps = spool.tile([128, TCH], f32, tag="s",
                                                      name="s_ps")
                                    nc.tensor.matmul(
                                        s_ps,
                                        kt_sb[:, h, i * 128 : (i + 1) * 128],
                                        qt_sb[:, h, qsl],
                                        start=True,
                                        stop=True,
                                    )
                                    a_t = apool.tile([128, TCH], cdt, tag="a",
                                                     name="a_t")
                                    nc.scalar.activation(a_t, s_ps, AF.Exp,
                                                         scale=SCALE)
                                    j = i - 4 * qc
                                    if j >= 0:
                                        nc.vector.tensor_mul(
                                            out=a_t, in0=a_t,
                                            in1=mask_sb[:, j * TCH : (j + 1) * TCH],
                                        )
                                    a_tiles[i] = a_t
                                ts = i - LA
                                if ts >= 0:
                                    a_t = a_tiles.pop(ts)
                                    nc.tensor.matmul(
                                        rs_ps, ones_col, a_t,
                                        start=(ts == 0), stop=(ts == n_ts - 1),
                                    )
                                    nc.tensor.matmul(
                                        o_ps,
                                        vnat[:, ts, h * 128 : (h + 1) * 128],
                                        a_t,
                                        start=(ts == 0), stop=(ts == n_ts - 1),
                                    )
                            # free the o_ps bank ASAP: copy to SBUF before the
                            # (slow) reciprocal/broadcast chain
                            of32 = rbbpool.tile([128, TCH], f32, tag="of32",
                                                name="of32")
                            nc.vector.tensor_copy(out=of32, in_=o_ps)
                            rinv = rbpool.tile([1, TCH], f32, tag="rinv",
                                               name="rinv")
                            nc.vector.reciprocal(rinv, rs_ps)
                            # broadcast rinv across partitions via PE
                            bb_ps = bbpool.tile([128, TCH], f32, tag="bb",
                                                name="bb_ps")
                            nc.tensor.matmul(bb_ps, ones_row, rinv,
                                             start=True, stop=True)
                            rbb = rbbpool.tile([128, TCH], f32, tag="rbb",
                                               name="rbb")
                            nc.vector.tensor_copy(out=rbb, in_=bb_ps)
                            ot = otpool.tile([128, TCH], cdt, tag="ot", name="ot")
                            nc.vector.tensor_mul(out=ot, in0=of32, in1=rbb)
                            nc.sync.dma_start(
                                out=a2a_in_v[:, qc * nj : (qc + 1) * nj, :],
                                in_=ot.rearrange("p (j tl) -> p j tl", tl=TSL),
                            )
                        nc.gpsimd.collective_compute(
                            "AllToAll",
                            mybir.AluOpType.bypass,
                            ins=[a2a_ins[h].opt()],
                            outs=[a2a_outs[h].opt()],
                            replica_groups=[list(range(NCORES))],
                        )

                # ---------------- phase 4: output projection ----------------
                # global out-channel tile k = HPC*j + h  (j = source core)
                with (
                    tc.tile_pool(name="otsb", bufs=1) as otsbp,
                    tc.tile_pool(name="y_ps", bufs=1, space="PSUM") as ypsp,
                    tc.tile_pool(name="ysb", bufs=3) as ysbp,
                ):
                    ot_sbs = []
                    for h in range(HPC):
                        ot_sb = otsbp.tile([128, NCORES, TSL], cdt,
                                           tag=f"ot_sb{h}", name=f"ot_sb{h}")
                        nc.sync.dma_start(
                            out=ot_sb,
                            in_=a2a_outs[h]
                            .rearrange("j ch tl -> (j ch) tl")
                            .rearrange("(k p) tl -> p k tl", p=128),
                        )
                        ot_sbs.append(ot_sb)
                    yps = {
                        (tt, jc): ypsp.tile(
                            [128, TCH], f32, tag=f"y{tt}{jc}",
                            name=f"y{tt}{jc}",
                        )
                        for tt in range(n_tt)
                        for jc in range(C // TCH)
                    }
                    for h in range(HPC):
                        for j8 in range(NCORES):
                            k = HPC * j8 + h
                            for tt in range(n_tt):
                                for jc in range(C // TCH):
                                    jsl = slice(jc * TCH, (jc + 1) * TCH)
                                    nc.tensor.matmul(
                                        yps[(tt, jc)],
                                        ot_sbs[h][:, j8, tt * 128 : (tt + 1) * 128],
                                        wp_res[k][:, jsl],
                                        start=(h == 0 and j8 == 0),
                                        stop=(h == HPC - 1 and j8 == NCORES - 1),
                                    )
                    for jc in range(C // TCH):
                        jsl = slice(jc * TCH, (jc + 1) * TCH)
                        for tt in range(n_tt):
                            y_sb = ysbp.tile([128, TCH], f32, tag="y_sb",
                                             name="y_sb")
                            nc.vector.tensor_add(
                                out=y_sb, in0=yps[(tt, jc)], in1=bias_bc[:, jsl]
                            )
                            nc.sync.dma_start(
                                out=y[tt * 128 : (tt + 1) * 128, jsl], in_=y_sb
                            )
                wpres_ctx.__exit__(None, None, None)

    nc.compile()
    return nc


def _perm_deinterleave():
    """Per 128-channel head block: [even channels..., odd channels...]."""
    perm = []
    for hl in range(HPC):
        base = hl * D
        perm.extend(base + 2 * np.arange(64))
        perm.extend(base + 2 * np.arange(64) + 1)
    return np.array(perm, dtype=np.int64)


def _shard_inputs(x, wq, bq, wk, bk, wv, bv, wp, bp, cos, sin, T, cdt_name):
    import ml_dtypes

    mask_np_dt = ml_dtypes.bfloat16 if cdt_name == "bfloat16" else np.float32
    perm = _perm_deinterleave()
    xT = np.ascontiguousarray(x.T)
    # rope tables duplicated across both partition halves so every DVE
    # 2-input op sees matching base partitions
    cosT = np.ascontiguousarray(np.concatenate([cos.T, cos.T], axis=0))
    sinT = np.ascontiguousarray(np.concatenate([sin.T, sin.T], axis=0))
    wpT = np.ascontiguousarray(wp.T)

    # masks[j]: allowed(ts_local=j*128+p, tq_local=f) for the 4 tiles of the
    # diagonal 512-wide chunk: keep if j*128 + p <= f  (0/1, exact in bf16)
    msk = np.zeros((128, 4 * TCH), dtype=np.float32)
    p = np.arange(128)[:, None]
    f = np.arange(TCH)[None, :]
    for j in range(4):
        msk[:, j * TCH : (j + 1) * TCH] = (j * 128 + p <= f).astype(np.float32)
    msk = msk.astype(mask_np_dt)

    in_maps = []
    for c in range(NCORES):
        sl = slice(c * M, (c + 1) * M)
        wq_c = wq[sl][perm]
        wk_c = wk[sl][perm]
        in_maps.append(
            {
                "xT": xT,
                "wqT": np.ascontiguousarray(wq_c.T),
                "wkT": np.ascontiguousarray(wk_c.T),
                "wvT": np.ascontiguousarray(wv[sl].T),
                "bq": np.ascontiguousarray(bq[sl][perm]),
                "bk": np.ascontiguousarray(bk[sl][perm]),
                "bv": np.ascontiguousarray(bv[sl]),
                "cosT": cosT,
                "sinT": sinT,
                "masks": msk,
                "wpT": wpT,
                "bp": bp,
            }
        )
    return in_maps


def kernel(x, wq, bq, wk, bk, wv, bv, wp, bp, cos, sin, trace=False):
    from concourse.bass_utils import run_bass_kernel_spmd

    T = x.shape[0]
    cdt_name = os.environ.get("MM_DT", "bfloat16")
    key = (T, cdt_name)
    if key not in _COMPILED:
        _COMPILED[key] = _build_nc(T, key[1])
    nc = _COMPILED[key]

    in_maps = _shard_inputs(
        x, wq, bq, wk, bk, wv, bv, wp, bp, cos, sin, T, cdt_name
    )
    res = run_bass_kernel_spmd(nc, in_maps, list(range(NCORES)), trace=trace)
    out = np.concatenate([res.results[c]["y"] for c in range(NCORES)], axis=0)
    kernel.last_result = res
    return out


if __name__ == "__main__":
    # smoke: build only
    _build_nc(int(os.environ.get("T", "2048")),
              os.environ.get("MM_DT", "bfloat16"))
    print("build ok")
